# revision 43
# baseline (speedup 1.0000x reference)
"""BasicSSM Trainium2 kernel.

Math: A_bar = expm(delta*A); u = x @ (delta*B)^T; h_t = h_{t-1} @ A_bar^T + u_t;
y = h @ C^T.

Because A = 0.05*randn - 0.5*I (documented construction in the reference), the
spectral radius of P = A_bar^T is ~0.65, so P^d decays below f32 resolution by
d ~ 64.  The scan is therefore computed as a windowed convolution
    H[s] = sum_{d=0}^{W-1} u[s-d] @ P^d          (W = 8*N_D0 lags)
which is exact to ~1e-8 relative and makes sequence sharding communication-free
(each core only needs a W-row halo of x).

Sharding: 8 cores = 4 batches x 2 sequence halves (communication-free).

I/O precision: x is downcast to bf16 AND pre-transposed on the host into a
chunk-major layout xts[p, NCH*r0(s) + c*rn + j] = x[r0(s)+j, c*128+p], so the
device does no PE transposes and reads half the bytes; u/u8/P-stacks are bf16;
H and C^T stay f32(r); y is stored bf16 and upcast on the host.  End-to-end
relative error ~3.5e-3 against the f32 reference (tolerance 2e-2); set
X_BF16=False for f32r I/O at ~1.5-2x the runtime.

Per core (2176 u columns = 128 halo + 2048 rows), spans aligned to the halo
grid (128 + 4x512) so scan window w needs exactly spans 0..w+1 with no
forward overreach:
  stage 1: two contiguous DMAs per span (halves, so the first 4 chunk-
           matmuls start at half-load) -> 8 accumulating matmuls
           (bbt chunk [128,16] x xts chunk [128,512]) -> u^T master (16, 2176)
  stage 2: per 512-col window, ONE overlapping-AP SBUF->SBUF DMA builds an
           8-lag-stacked tile u8[(m,d_rev), j] = u^T[m, base+d_rev+j]; 8
           accumulating matmuls against host-built P-power stacks -> H^T
  stage 3: y window (512,1024) = 8 matmuls H^T_slice.T @ C^T (PSUM) -> bf16
           copy into a [128,4,1024] supertile -> two 512KB DMAs to HBM
Scan/projection PE compute uses float32r / bf16 (full matmul rate).  Stages
are interleaved per-window; DMA roles are split across queues: SP/sync (x
loads), ACT (y stores + psh copies), GPSIMD/SWDGE (u8 builds) to avoid
per-ring FIFO head-of-line blocking.  u^T/H^T masters are double-buffered
per iteration, and the bench loop statically unrolls 4 iterations per For_i
trip, so consecutive iterations overlap (loads of k+1 under compute of k).
"""

import numpy as np

D_MODEL = 1024
D_STATE = 16
BATCH = 4
SEQ = 4096
N_CORES = 8
HALF = SEQ // 2           # 2048 rows of output per core
HP = 128                  # halo rows (supports window up to 128 lags)
ROWS = HP + HALF          # 2176 = 17 tiles of 128
NW = HALF // 512          # 4 scan windows of 512
NCH = D_MODEL // 128      # 8 contraction chunks of 128
N_D0 = 8                  # 8-lag groups -> window W = 64 lags (may be
                          # widened at run time if P decays slowly; the
                          # HP=128 halo supports N_D0 <= 15)
U8F = 512 + 8 * N_D0 - 1  # u8 tile free size (575)
LM = 8 * N_D0 - 1         # left margin inside u8 tile (63)
# spans aligned to the halo grid: window w needs exactly spans 0..w+1 and
# span w+1 ends at window w's last needed u column (no forward overreach)
SPANS = [(0, HP)] + [(HP + i * 512, 512) for i in range(4)]


def _set_window(n_d0):
    global N_D0, U8F, LM
    N_D0 = n_d0
    U8F = 512 + 8 * N_D0 - 1
    LM = 8 * N_D0 - 1

_CACHE = {}
LAST_RESULTS = None  # BassKernelResults from the most recent run (for profiling)
TRACE = False
X_BF16 = True  # upload x (and store y) as bf16: half the HBM traffic


def _expm(M):
    """Scaling-and-squaring Taylor expm in float64 (16x16, ||M|| ~ 0.7)."""
    M = np.asarray(M, dtype=np.float64)
    nrm = np.linalg.norm(M, 1)
    s = max(0, int(np.ceil(np.log2(max(nrm, 1e-300)))) + 1) if nrm > 0.5 else 0
    Ms = M / (2.0 ** s)
    E = np.eye(M.shape[0])
    T = np.eye(M.shape[0])
    for k in range(1, 40):
        T = T @ Ms / k
        E = E + T
    for _ in range(s):
        E = E @ E
    return E


def _build_program(loop_n=None, static_unroll=1):
    """Build the (shared, SPMD) Bass program once.  loop_n!=None builds a
    timing variant: body repeated static_unroll times inside a hardware
    For_i loop, xts/ys internal DRAM (garbage data, tiny external I/O) so
    dispatch cost is negligible."""
    import concourse.bass as bass
    import concourse.bacc as bacc
    import concourse.mybir as mybir
    import concourse.tile as tile

    f32 = mybir.dt.float32
    f32r = mybir.dt.float32r
    bf16 = mybir.dt.bfloat16
    dtx = bf16 if X_BF16 else f32r  # x / y HBM dtype
    nc = bacc.Bacc(
        "TRN2", target_bir_lowering=False, debug=False, num_devices=N_CORES
    )

    XF = NCH * ROWS  # 17408 free elements in the transposed-x layout
    if loop_n is None:
        xts = nc.dram_tensor("xts", [128, XF], dtx, kind="ExternalInput")
        ys = nc.dram_tensor("ys", [HALF, D_MODEL], dtx, kind="ExternalOutput")
    else:
        xts = nc.dram_tensor("xts", [128, XF], dtx)
        ys = nc.dram_tensor("ys", [HALF, D_MODEL], dtx)
        done = nc.dram_tensor("done", [128, 1], dtx, kind="ExternalOutput")
    bbt = nc.dram_tensor("bbt", [128, NCH * D_STATE], dtx, kind="ExternalInput")
    pc = nc.dram_tensor("pc", [128, N_D0 * D_STATE], dtx, kind="ExternalInput")
    ct = nc.dram_tensor("ct", [D_STATE, D_MODEL], f32r, kind="ExternalInput")

    with tile.TileContext(nc) as tc:
        with (
            tc.tile_pool(name="consts", bufs=1) as consts,
            tc.tile_pool(name="xt", bufs=5) as xtp,
            tc.tile_pool(name="masters", bufs=4) as masters,
            tc.tile_pool(name="u8", bufs=4) as u8p,
            tc.tile_pool(name="yout", bufs=4) as youtp,
            tc.tile_pool(name="ps_u", bufs=2, space=bass.MemorySpace.PSUM) as ps_u,
            tc.tile_pool(name="ps_h", bufs=2, space=bass.MemorySpace.PSUM) as ps_h,
            tc.tile_pool(name="ps_y", bufs=4, space=bass.MemorySpace.PSUM) as ps_y,
        ):
            # --- constants ---
            bbt_s = consts.tile([128, NCH * D_STATE], dtx)
            nc.scalar.dma_start(bbt_s[:], bbt[:])
            pc_s = consts.tile([128, N_D0 * D_STATE], dtx)
            nc.scalar.dma_start(pc_s[:], pc[:])
            ct_s = consts.tile([D_STATE, D_MODEL], f32r)
            nc.scalar.dma_start(ct_s[:], ct[:])

            # u^T / H^T masters are allocated per schedule() call (bufs=2)
            # so consecutive unrolled iterations alternate buffers and
            # decouple: iteration k+1's stage-1 writes need not wait for
            # iteration k's scan/projection readers.
            state = {}

            # stage-1 span: u^T[:, r0:r0+rn] accumulated over 8 chunks;
            # the load is split in two so the first 4 chunk-matmuls can
            # start at half-load (range-granular deps)
            def st(i):
                r0, rn = SPANS[i]
                xi = xtp.tile([128, NCH * 512], dtx, tag="xt")
                if rn > 128:
                    h = NCH * rn // 2
                    nc.sync.dma_start(
                        xi[:, :h], xts[:, NCH * r0:NCH * r0 + h]
                    )
                    nc.sync.dma_start(
                        xi[:, h:NCH * rn], xts[:, NCH * r0 + h:NCH * (r0 + rn)]
                    )
                else:
                    nc.sync.dma_start(
                        xi[:, :NCH * rn], xts[:, NCH * r0:NCH * (r0 + rn)]
                    )
                psu = ps_u.tile([D_STATE, 512], f32, tag="psu")
                for c in range(NCH):
                    nc.tensor.matmul(
                        psu[:, :rn],
                        bbt_s[:, c * D_STATE:(c + 1) * D_STATE],
                        xi[:, c * rn:(c + 1) * rn],
                        start=(c == 0),
                        stop=(c == NCH - 1),
                    )
                nc.vector.tensor_copy(state["utm"][:, r0:r0 + rn], psu[:, :rn])

            # stage-2 window: H^T[:, 512w:512w+512] (windowed scan)
            def win(w):
                w0 = HP + 512 * w
                u8 = u8p.tile([128, U8F], dtx, tag="u8")
                # one DMA builds all 8 shifted copies: in-AP dims
                # [d_rev: +1 col, 8][n: +row, 16][j: +1, U8F] (overlapping
                # reads; d reversed so the shift step is positive; the d
                # reversal is baked into pc on the host)
                utm_base = state["utm"][:, 0:1]
                src = bass.AP(
                    utm_base.tensor,
                    utm_base.offset + (w0 - LM - 7),
                    [[ROWS, D_STATE], [1, 8], [1, U8F]],
                )
                nc.gpsimd.dma_start(u8[:], src)
                psh = ps_h.tile([D_STATE, 512], f32, tag="psh")
                for d0 in range(N_D0):
                    off = LM - 8 * d0  # rhs col j' reads u at lag 8*d0+d
                    nc.tensor.matmul(
                        psh[:],
                        pc_s[:, d0 * D_STATE:(d0 + 1) * D_STATE],
                        u8[:, off:off + 512],
                        start=(d0 == 0),
                        stop=(d0 == N_D0 - 1),
                    )
                nc.vector.tensor_copy(state["htm"][:, w * 512:(w + 1) * 512], psh[:])

            # stage-3 y window: y[512w:512w+512, :] = H @ C^T, stored bf16
            def ywin(w):
                yts = youtp.tile([128, 4, D_MODEL], dtx, tag="yts")
                for q in range(4):
                    t = 4 * w + q
                    for g in range(2):
                        psy = ps_y.tile([128, 512], f32, tag="psy")
                        nc.tensor.matmul(
                            psy[:],
                            state["htm"][:, t * 128:(t + 1) * 128],
                            ct_s[:, g * 512:(g + 1) * 512],
                            start=True,
                            stop=True,
                        )
                        if g == 0:
                            nc.vector.tensor_copy(yts[:, q, :512], psy[:])
                        else:
                            nc.scalar.copy(yts[:, q, 512:], psy[:])
                dst = ys[w * 512:(w + 1) * 512, :].rearrange(
                    "(q p) j -> p q j", p=128
                )
                nc.scalar.dma_start(dst, yts[:])

            # interleaved schedule: window w needs supertiles 0..w+1
            def schedule():
                state["utm"] = masters.tile(
                    [D_STATE, ROWS], dtx, tag="utm", name="utm"
                )
                state["htm"] = masters.tile(
                    [D_STATE, HALF], f32r, tag="htm", name="htm"
                )
                st(0)
                st(1)
                win(0)
                ywin(0)
                st(2)
                win(1)
                ywin(1)
                st(3)
                win(2)
                ywin(2)
                st(4)
                win(3)
                ywin(3)

            if loop_n is None:
                for _ in range(static_unroll):
                    schedule()
            else:
                assert loop_n % static_unroll == 0
                with tc.For_i(0, loop_n // static_unroll, 1):
                    for _ in range(static_unroll):
                        schedule()
                nc.sync.dma_start(done[:], pc_s[:, 0:1])

    nc.compile()
    return nc


def _get_runner(nc):
    """Cached shard_map runner (mirrors bass2jax.run_bass_via_pjrt but the
    jitted callable persists across kernel() calls)."""
    import jax
    import numpy as _np
    from jax.sharding import Mesh, PartitionSpec
    try:
        from jax.experimental.shard_map import shard_map
    except ImportError:
        from jax.shard_map import shard_map
    import concourse.mybir as mybir
    from concourse import bass2jax

    bass2jax.install_neuronx_cc_hook()
    part_name = nc.partition_id_tensor.name if nc.partition_id_tensor else None
    in_names, out_names, out_avals, zero_outs = [], [], [], []
    for alloc in nc.m.functions[0].allocations:
        if not isinstance(alloc, mybir.MemoryLocationSet):
            continue
        name = alloc.memorylocations[0].name
        if alloc.kind == "ExternalInput":
            if name != part_name:
                in_names.append(name)
        elif alloc.kind == "ExternalOutput":
            shape = tuple(alloc.tensor_shape)
            dtype = mybir.dt.np(alloc.dtype)
            out_names.append(name)
            out_avals.append(jax.core.ShapedArray(shape, dtype))
            zero_outs.append(_np.zeros(shape, dtype))
    n_params = len(in_names)
    n_outs = len(out_avals)
    all_names = in_names + out_names
    if part_name is not None:
        all_names = all_names + [part_name]
    donate = tuple(range(n_params, n_params + n_outs))

    def _body(*args):
        operands = list(args)
        if part_name is not None:
            operands.append(bass2jax.partition_id_tensor())
        outs = bass2jax._bass_exec_p.bind(
            *operands,
            out_avals=tuple(out_avals),
            in_names=tuple(all_names),
            out_names=tuple(out_names),
            lowering_input_output_aliases=(),
            sim_require_finite=True,
            sim_require_nnan=True,
            nc=nc,
        )
        return tuple(outs)

    devices = jax.devices()[:N_CORES]
    mesh = Mesh(np.asarray(devices), ("core",))
    specs = (PartitionSpec("core"),) * (n_params + n_outs)
    sharded = jax.jit(
        shard_map(_body, mesh=mesh, in_specs=specs,
                  out_specs=(PartitionSpec("core"),) * n_outs, check_rep=False),
        donate_argnums=donate, keep_unused=True,
    )
    return sharded, in_names, out_names, zero_outs


def _run_spmd_cached(nc, in_maps):
    import jax
    if "runner" not in _CACHE:
        _CACHE["runner"] = _get_runner(nc)
    sharded, in_names, out_names, zero_outs = _CACHE["runner"]
    concat_in = [
        np.concatenate([np.asarray(in_maps[c][n]) for c in range(N_CORES)], axis=0)
        for n in in_names
    ]
    concat_zero = [np.concatenate([z] * N_CORES, axis=0) for z in zero_outs]
    outs = sharded(*concat_in, *concat_zero)
    outs = [np.asarray(o) for o in outs]
    results = []
    for c in range(N_CORES):
        m = {}
        for i, n in enumerate(out_names):
            per = outs[i].shape[0] // N_CORES
            m[n] = outs[i][c * per:(c + 1) * per]
        results.append(m)
    return results


def bench_hw(x, A, B, C, delta, n=2048, n0=1024):
    """Absolute HW timing via a For_i-looped variant of the program with
    internal xts/ys (tiny external I/O).  Returns (times, per_iter_seconds)."""
    import time as _time
    import jax
    kernel(x, A, B, C, delta)  # fills _CACHE["last_in_maps"]
    in_maps = _CACHE["last_in_maps"]

    results = {}
    for n_iter in (n0, n):
        key = f"loopnc_{n_iter}"
        if key not in _CACHE:
            _CACHE[key] = _build_program(loop_n=n_iter, static_unroll=4)
            _CACHE[key + "_runner"] = _get_runner(_CACHE[key])
        ncl = _CACHE[key]
        sharded, in_names, out_names, zero_outs = _CACHE[key + "_runner"]
        concat_in = [
            np.concatenate(
                [np.asarray(in_maps[c][nm]) for c in range(N_CORES)], axis=0
            )
            for nm in in_names
        ]
        best = 1e9
        for rep in range(8):
            concat_zero = [np.concatenate([z] * N_CORES, axis=0) for z in zero_outs]
            t0 = _time.time()
            r = sharded(*concat_in, *concat_zero)
            jax.block_until_ready(r)
            dt = _time.time() - t0
            if rep > 0:
                best = min(best, dt)
        results[n_iter] = best
    per_iter = (results[n] - results[n0]) / (n - n0)
    return results, per_iter


def _np_dtx():
    import concourse.mybir as mybir
    if X_BF16:
        return mybir.dt.np(mybir.dt.bfloat16)
    return np.float32


def kernel(x, A, B, C, delta):
    global LAST_RESULTS
    from concourse.bass_utils import run_bass_kernel_spmd

    x = np.ascontiguousarray(np.asarray(x, dtype=np.float32))
    dl = float(np.asarray(delta).reshape(-1)[0])
    np_dtx = _np_dtx()

    # host-side tiny-weight prep (float64)
    A_bar = _expm(dl * np.asarray(A, np.float64))       # (N, N)
    P = A_bar.T
    pows = [np.eye(D_STATE)]
    for _ in range(8 * 15):
        pows.append(pows[-1] @ P)
    # widen the window if P^(8*N_D0) hasn't decayed below f32 significance
    want = 8
    while want < 15 and np.linalg.norm(pows[8 * want], 2) > 1e-7:
        want += 1
    if want != N_D0:
        _set_window(want)
        _CACHE.clear()
    # u8 partition layout is (m, d_rev) = m*8 + d_rev (partition-major DMA
    # legality) with d reversed so the shift step is +1; pc rows match:
    # pc[m*8 + dr, d0*16 + n] = P^(8*d0 + 7 - dr)[m, n]
    pc_np = np.zeros((128, N_D0 * D_STATE), np.float32)
    for d0 in range(N_D0):
        for dr in range(8):
            for m in range(D_STATE):
                pc_np[m * 8 + dr, d0 * D_STATE:(d0 + 1) * D_STATE] = \
                    pows[8 * d0 + 7 - dr][m].astype(np.float32)
    pc_np = np.ascontiguousarray(pc_np.astype(np_dtx))
    # bbt[p, c*16+n] = (delta*B)[n, c*128+p]
    Bb = (dl * np.asarray(B, np.float64)).astype(np.float32)  # (16, 1024)
    bbt_np = np.ascontiguousarray(
        Bb.T.reshape(NCH, 128, D_STATE).transpose(1, 0, 2).reshape(
            128, NCH * D_STATE
        ).astype(np_dtx)
    )
    ct_np = np.ascontiguousarray(np.asarray(C, np.float32).T)

    if "nc" not in _CACHE:
        _CACHE["nc"] = _build_program()
    nc = _CACHE["nc"]
    assert np.linalg.norm(pows[8 * N_D0], 2) <= 1e-6, "window too short for this A"

    in_maps = []
    for core in range(N_CORES):
        b, half = divmod(core, 2)
        t0 = half * HALF
        xs_np = np.zeros((ROWS, D_MODEL), np.float32)
        if t0 >= HP:
            xs_np[:HP] = x[b, t0 - HP:t0]
        xs_np[HP:] = x[b, t0:t0 + HALF]
        # transposed chunk-major layout: xt[p, NCH*r0 + c*rn + j]
        #   = x_slice[r0+j, c*128+p]
        Xc = xs_np.T.reshape(NCH, 128, ROWS)
        parts = [
            np.transpose(Xc[:, :, r0:r0 + rn], (1, 0, 2)).reshape(128, NCH * rn)
            for (r0, rn) in SPANS
        ]
        xt_np = np.ascontiguousarray(
            np.concatenate(parts, axis=1).astype(np_dtx)
        )
        in_maps.append({
            "xts": xt_np, "bbt": bbt_np, "pc": pc_np, "ct": ct_np,
        })

    _CACHE["last_in_maps"] = in_maps
    if TRACE:
        res = run_bass_kernel_spmd(nc, in_maps, list(range(N_CORES)), trace=True)
        LAST_RESULTS = res
        results = res.results
    else:
        results = _run_spmd_cached(nc, in_maps)

    y = np.empty((BATCH, SEQ, D_MODEL), np.float32)
    for core in range(N_CORES):
        b, half = divmod(core, 2)
        y[b, half * HALF:(half + 1) * HALF, :] = \
            np.asarray(results[core]["ys"]).astype(np.float32)
    return y
